# revision 7
# baseline (speedup 1.0000x reference)
"""BitLinear 1-bit (BitNet-style) linear layer on 8 Trainium2 NeuronCores.

y = x_q @ Wb^T where
  x_q = per-token group-64 absmax int8 fake-quant of x
  Wb  = per-row centered binarization: sign(W - rowmean) * rowmean(|W - rowmean|)

Sharding: data-parallel over tokens. Each core gets a 1024-token slice of x
(full 4096-feature rows) plus the full W, computes its y^T shard
[4096 out, 1024 tok] so the per-row alpha scale is a per-partition scalar,
and the host concatenates + transposes.

Key kernel choices:
  - matmul runs in bf16: the weight side is exactly +-1 (exact in bf16, alpha
    factored out of the matmul and applied at PSUM eviction); only x_q is
    rounded to bf16 (measured ~1.2e-3 absmax relative error vs f32 reference).
  - round() is the f32 magic-number trick (+1.5*2^23, -1.5*2^23 = RNE),
    run on GPSIMD as a dual-op tensor_scalar. clip is a no-op since
    |x/scale*127| <= 127 by construction.
  - both x_q^T and S^T = sign(W-m)^T are produced by PE transposes (bf16,
    1 cycle/row) into PSUM, evicted to SBUF by ACT/DVE copies.
"""

import sys

sys.path.insert(0, "/opt/trn_rl_repo")

import numpy as np

import concourse.bacc as bacc
import concourse.tile as tile
from concourse import mybir
from concourse.bass_utils import run_bass_kernel_spmd
from concourse.masks import make_identity

F32 = mybir.dt.float32
BF16 = mybir.dt.bfloat16
AX = mybir.AxisListType
ALU = mybir.AluOpType
ACTF = mybir.ActivationFunctionType

MAGIC = 1.5 * 2**23  # adding+subtracting forces RNE round-to-integer in f32
QMAX = 127.0
EPS = 1e-8
GROUP = 64

N_CORES = 8
B, S, D_IN, D_OUT = 4, 2048, 4096, 4096
T_TOTAL = B * S


def build_program(T=1024, D=4096, O=4096, tchunk=512):
    """Emit the per-core program. T tokens x [O, D] weight -> yT [O, T]."""
    P = 128
    nt = T // P          # token tiles
    nk = D // P          # contraction (k) blocks
    no = O // P          # output-row tiles
    ntc = T // tchunk    # token chunks per matmul sweep
    ng = D // GROUP      # quant groups per token row
    kb = 4               # k-blocks transposed per PSUM bank (4*128 = 512 cols)

    nc = bacc.Bacc(None, target_bir_lowering=False)

    x_d = nc.dram_tensor("x", [T, D], F32, kind="ExternalInput")
    w_d = nc.dram_tensor("W", [O, D], F32, kind="ExternalInput")
    y_d = nc.dram_tensor("yT", [O, T], F32, kind="ExternalOutput")

    with tile.TileContext(nc) as tc:
        with (
            tc.tile_pool(name="const", bufs=1) as constp,
            tc.tile_pool(name="xin", bufs=2) as xinp,
            tc.tile_pool(name="qs", bufs=2) as qsp,
            tc.tile_pool(name="sc", bufs=3) as scp,
            tc.tile_pool(name="xqt", bufs=1) as xqtp,
            tc.tile_pool(name="win", bufs=2) as winp,
            tc.tile_pool(name="sgn", bufs=2) as sgnp,
            tc.tile_pool(name="st", bufs=2) as stp,
            tc.tile_pool(name="wsc", bufs=3) as wscp,
            tc.tile_pool(name="yout", bufs=2) as youtp,
            tc.tile_pool(name="tp", bufs=4, space="PSUM") as tpp,
            tc.tile_pool(name="yp", bufs=2, space="PSUM") as ypp,
        ):
            ident = constp.tile([P, P], BF16)
            make_identity(nc, ident[:])

            # x_q^T stays resident in SBUF: [128, nk, T] bf16 (d on partitions)
            xqt = xqtp.tile([P, nk, T], BF16)

            # ---- Phase A: quantize + transpose the x slice, one 128-token tile at a time
            x_t = x_d.rearrange("(n p) d -> n p d", p=P)
            for t in range(nt):
                xt = xinp.tile([P, D], F32, tag="xt")
                nc.sync.dma_start(xt[:], x_t[t])
                xg = xt[:].rearrange("p (g e) -> p g e", e=GROUP)

                # group absmax -> scale; sr = max(scale,eps)/127 ; rs = 1/sr
                amax = scp.tile([P, ng], F32, tag="amax")
                nc.vector.tensor_reduce(
                    amax[:], xg, axis=AX.X, op=ALU.max, apply_absolute_value=True
                )
                sr = scp.tile([P, ng], F32, tag="sr")
                nc.vector.tensor_scalar(
                    sr[:], amax[:], EPS, 1.0 / QMAX, op0=ALU.max, op1=ALU.mult
                )
                rs = scp.tile([P, ng], F32, tag="rs")
                nc.vector.reciprocal(rs[:], sr[:])

                # t1 = x * rs (broadcast rs over the 64-wide group), in place
                rs_b = rs[:].unsqueeze(-1).broadcast_to((P, ng, GROUP))
                nc.vector.tensor_tensor(xg, xg, rs_b, op=ALU.mult)
                # q = RNE-round(t1) via magic number; q in [-127,127] ints, exact in bf16
                q = qsp.tile([P, D], BF16, tag="q")
                nc.vector.tensor_scalar(
                    q[:], xt[:], MAGIC, MAGIC, op0=ALU.add, op1=ALU.subtract
                )
                # xq = q * sr -> bf16, in place over q
                qg = q[:].rearrange("p (g e) -> p g e", e=GROUP)
                sr_b = sr[:].unsqueeze(-1).broadcast_to((P, ng, GROUP))
                nc.vector.tensor_tensor(qg, qg, sr_b, op=ALU.mult)

                # transpose 128x128 blocks via PE into PSUM (4 per bank), evict
                for m in range(nk // kb):
                    tp = tpp.tile([P, kb * P], BF16, tag="tp")
                    for j in range(kb):
                        k = m * kb + j
                        nc.tensor.transpose(
                            tp[:, j * P : (j + 1) * P],
                            q[:, k * P : (k + 1) * P],
                            ident[:],
                        )
                    dst = xqt[:, m * kb : (m + 1) * kb, t * P : (t + 1) * P]
                    src = tp[:].rearrange("p (j c) -> p j c", c=P)
                    nc.scalar.copy(dst, src)

            # ---- Phase B: per 128-row weight tile: binarize, transpose, matmul
            w_t = w_d.rearrange("(n p) d -> n p d", p=P)
            y_t = y_d.rearrange("(n p) t -> n p t", p=P)
            for o in range(no):
                wt = winp.tile([P, D], F32, tag="wt")
                nc.sync.dma_start(wt[:], w_t[o])

                sg = sgnp.tile([P, D], BF16, tag="sg")
                # row mean
                msum = wscp.tile([P, 1], F32, tag="msum")
                nc.vector.tensor_reduce(msum[:], wt[:], axis=AX.X, op=ALU.add)
                negm = wscp.tile([P, 1], F32, tag="negm")
                nc.vector.tensor_scalar(
                    negm[:], msum[:], -1.0 / D, None, op0=ALU.mult
                )
                # alpha = mean(|W - m|) via ACT Abs with accum (sg is scratch again)
                asum = wscp.tile([P, 1], F32, tag="asum")
                nc.scalar.activation(
                    sg[:], wt[:], ACTF.Abs, bias=negm[:, 0:1], accum_out=asum[:]
                )
                alpha = wscp.tile([P, 1], F32, tag="alpha")
                nc.vector.tensor_scalar(
                    alpha[:], asum[:], 1.0 / D, None, op0=ALU.mult
                )
                # S = sign(W - m) in {-1, +1}, exact in bf16
                nc.scalar.activation(sg[:], wt[:], ACTF.Sign, bias=negm[:, 0:1])

                # S^T for this o-tile: [128, nk, 128] bf16
                st = stp.tile([P, nk, P], BF16, tag="st")
                for m in range(nk // kb):
                    tp = tpp.tile([P, kb * P], BF16, tag="tp")
                    for j in range(kb):
                        k = m * kb + j
                        nc.tensor.transpose(
                            tp[:, j * P : (j + 1) * P],
                            sg[:, k * P : (k + 1) * P],
                            ident[:],
                        )
                    dst = st[:, m * kb : (m + 1) * kb, :]
                    src = tp[:].rearrange("p (j c) -> p j c", c=P)
                    nc.vector.tensor_copy(dst, src)

                # matmul sweep: yT[o-tile, :] = (S^T).T @ xq^T, k-accumulated
                for t2 in range(ntc):
                    yp = ypp.tile([P, tchunk], F32, tag="yp")
                    for k in range(nk):
                        nc.tensor.matmul(
                            yp[:],
                            st[:, k, :],
                            xqt[:, k, t2 * tchunk : (t2 + 1) * tchunk],
                            start=(k == 0),
                            stop=(k == nk - 1),
                        )
                    yo = youtp.tile([P, tchunk], F32, tag="yo")
                    # evict + fold in alpha (per-partition scale)
                    nc.scalar.activation(
                        yo[:], yp[:], ACTF.Copy, scale=alpha[:, 0:1]
                    )
                    nc.sync.dma_start(
                        y_d[o * P : (o + 1) * P, t2 * tchunk : (t2 + 1) * tchunk],
                        yo[:],
                    )

    nc.compile()
    return nc


_cached = {}

# set by test harnesses: when True, capture an NTFF trace of core 0 and stash
# the BassKernelResults (with exec_time_ns) in LAST_RESULTS.
TRACE = False
LAST_RESULTS = None


def _get_program(key):
    if key not in _cached:
        T, D, O, tchunk = key
        _cached[key] = build_program(T, D, O, tchunk)
    return _cached[key]


def kernel(x: np.ndarray, W: np.ndarray) -> np.ndarray:
    assert x.shape == (B, S, D_IN) and W.shape == (D_OUT, D_IN)
    x2 = np.ascontiguousarray(x.reshape(T_TOTAL, D_IN), dtype=np.float32)
    Wc = np.ascontiguousarray(W, dtype=np.float32)

    t_core = T_TOTAL // N_CORES
    nc = _get_program((t_core, D_IN, D_OUT, 512))

    in_maps = [
        {"x": x2[i * t_core : (i + 1) * t_core], "W": Wc} for i in range(N_CORES)
    ]
    global LAST_RESULTS
    res = run_bass_kernel_spmd(
        nc, in_maps, core_ids=list(range(N_CORES)), trace=TRACE
    )
    LAST_RESULTS = res

    y2 = np.empty((T_TOTAL, D_OUT), dtype=np.float32)
    for i in range(N_CORES):
        y2[i * t_core : (i + 1) * t_core, :] = res.results[i]["yT"].T
    return y2.reshape(B, S, D_OUT)
